# revision 2
# baseline (speedup 1.0000x reference)
"""Tensor-parallel multi-head attention for 8 Trainium2 NeuronCores.

Sharding: tensor-parallel over the 16 heads (2 heads = 128 Q/K/V output dims
per core). Each core computes its heads' Q/K/V projections, attention, and a
partial fc_out ( O_c @ Wo[:, c-slice].T ); the host sums the 8 partials and
adds the output bias (the "all-reduce after fc_out" done host-side).

Per-core dataflow (per batch):
  x^T tiles (bf16, pre-transposed on host) -> QT/KT/VT [128, S] via PE
  E^T = exp(K_h Q_h^T / 8) with both heads' QK matmuls paired (partition
  bases 0/64 run in separate PE quadrants) and 1024-wide Exp activations
  (amortizes the ACT engine's 352-cycle fixed cost per instruction).
  AV: out^T[65, sq] = [V_h | 1]^T.T @ E^T  — the appended ones-column yields
  the softmax denominators in row 64 for free.
  Normalize (reciprocal + partition-broadcast + multiply), then
  YT_partial = Wo_c^T.T @ O^T  staged and streamed to DRAM in bf16
  (halves the store traffic; the 8 partials are summed in fp32 on the host,
  adding ~0.4% quantization error against the 2e-2 budget).
"""

from contextlib import ExitStack

import numpy as np
import ml_dtypes

import concourse.bacc as bacc
import concourse.mybir as mybir
import concourse.tile as tile
from concourse.bass import ts
from concourse.masks import make_identity
from concourse.bass_utils import run_bass_kernel_spmd
from concourse.bass_interp import get_hw_module

FP32 = mybir.dt.float32
BF16 = mybir.dt.bfloat16

E = 1024
S = 2048
NB = 4
DL = 128  # per-core Q/K/V dims (2 heads x 64)
D = 64
HL = DL // D
NJ = E // 128
NO = E // 128
SQ = 512
NQ = S // SQ
NK = S // 128
NBS = NB * S
N_CORES = 8


def _build():
    nc = bacc.Bacc("TRN2", target_bir_lowering=False, debug=True)

    xt = nc.dram_tensor("xt", [E, NBS], BF16, kind="ExternalInput")
    wq = nc.dram_tensor("wq", [128, NJ, DL], BF16, kind="ExternalInput")
    wk = nc.dram_tensor("wk", [128, NJ, DL], BF16, kind="ExternalInput")
    wv = nc.dram_tensor("wv", [128, NJ, DL], BF16, kind="ExternalInput")
    wot = nc.dram_tensor("wot", [DL, NO, 128], BF16, kind="ExternalInput")
    bq = nc.dram_tensor("bq", [DL, 1], FP32, kind="ExternalInput")
    bk = nc.dram_tensor("bk", [DL, 1], FP32, kind="ExternalInput")
    bv = nc.dram_tensor("bv", [DL, 1], FP32, kind="ExternalInput")
    yt = nc.dram_tensor("yt", [E, NBS], BF16, kind="ExternalOutput")

    with tile.TileContext(nc) as tc, ExitStack() as ctx:
        const = ctx.enter_context(tc.tile_pool(name="const", bufs=1))
        wq_sb = const.tile([128, NJ, DL], BF16)
        wk_sb = const.tile([128, NJ, DL], BF16)
        wv_sb = const.tile([128, NJ, DL], BF16)
        wot_sb = const.tile([DL, NO, 128], BF16)
        bq_sb = const.tile([DL, 1], FP32)
        bk_sb = const.tile([DL, 1], FP32)
        bv_sb = const.tile([DL, 1], FP32)
        ident = const.tile([128, 64], BF16)
        nc.sync.dma_start(wq_sb[:], wq[:, :, :])
        nc.sync.dma_start(wk_sb[:], wk[:, :, :])
        nc.sync.dma_start(wv_sb[:], wv[:, :, :])
        nc.sync.dma_start(wot_sb[:], wot[:, :, :])
        nc.sync.dma_start(bq_sb[:], bq[:, :])
        nc.sync.dma_start(bk_sb[:], bk[:, :])
        nc.sync.dma_start(bv_sb[:], bv[:, :])
        make_identity(nc, ident[0:64, :])
        nc.vector.tensor_copy(ident[64:128, :], ident[0:64, :])

        xt_pool = ctx.enter_context(tc.tile_pool(name="xt_pool", bufs=2))
        qkv_pool = ctx.enter_context(tc.tile_pool(name="qkv_pool", bufs=2))
        v1_pool = ctx.enter_context(tc.tile_pool(name="v1_pool", bufs=2))
        et_pool = ctx.enter_context(tc.tile_pool(name="et_pool", bufs=2))
        ot_pool = ctx.enter_context(tc.tile_pool(name="ot_pool", bufs=2))
        nrm_pool = ctx.enter_context(tc.tile_pool(name="nrm_pool", bufs=3))
        stage_pool = ctx.enter_context(tc.tile_pool(name="stage_pool", bufs=4))
        # PSUM budget (8 banks): pp_mm [128,2,512]f32 slots = 2 banks x 2 bufs,
        # pp_av [128,512] x 4 bufs = 4 banks.  V-transposes share pp_mm slots.
        pp_mm = ctx.enter_context(tc.tile_pool(name="pp_mm", bufs=2, space="PSUM"))
        pp_av = ctx.enter_context(tc.tile_pool(name="pp_av", bufs=4, space="PSUM"))

        def batch_body(n):
            # ---- load xT for this batch (per-chunk tiles: projections can
            # start as each 128-dim contraction chunk arrives) ----
            xt_tiles = []
            for j in range(NJ):
                xj = xt_pool.tile([128, S], BF16, name=f"xt_sb{j}", tag=f"xt{j}")
                nc.sync.dma_start(
                    xj[:, :], xt[j * 128 : (j + 1) * 128, n * S : (n + 1) * S]
                )
                xt_tiles.append(xj)

            # ---- Q/K/V projections (transposed layout [DL, S]) ----
            qt_sb = qkv_pool.tile([DL, S], BF16, name="qt_sb")
            kt_sb = qkv_pool.tile([DL, S], BF16, name="kt_sb")
            vt_sb = qkv_pool.tile([DL, S], BF16, name="vt_sb")
            for w_sb, b_sb, dst in (
                (wq_sb, bq_sb, qt_sb),
                (wk_sb, bk_sb, kt_sb),
                (wv_sb, bv_sb, vt_sb),
            ):
                for nn2 in range(0, NQ, 2):
                    psw = pp_mm.tile([128, 2, SQ], FP32, name="ps_projw", tag="mm")
                    for u in range(2):
                        for j in range(NJ):
                            nc.tensor.matmul(
                                psw[:DL, u, :],
                                w_sb[:, j, :],
                                xt_tiles[j][:, ts(nn2 + u, SQ)],
                                start=(j == 0),
                                stop=(j == NJ - 1),
                            )
                    nc.vector.tensor_scalar_add(
                        dst[:, nn2 * SQ : (nn2 + 2) * SQ], psw[:DL, :, :], b_sb[:, :]
                    )

            # ---- V^T -> [V | 1] in natural layout, per head ----
            v1_sbs = []
            for h in range(HL):
                v1_sb = v1_pool.tile([128, NK, D + 1], BF16, name=f"v1_sb_{h}")
                nc.gpsimd.memset(v1_sb[:, :, D : D + 1], 1.0)
                v1_sbs.append(v1_sb)
            for ck in range(NK):
                pts = []
                for h in range(HL):
                    pt = pp_mm.tile([128, D], BF16, name="pt_vt", tag="mm")
                    nc.tensor.transpose(
                        pt[:, :],
                        vt_sb[h * D : (h + 1) * D, ts(ck, 128)],
                        ident[h * D : (h + 1) * D, :],
                    )
                    pts.append(pt)
                for h in range(HL):
                    nc.vector.tensor_copy(v1_sbs[h][:, ck, 0:D], pts[h][:, :])

            # ---- attention (heads paired on PE quadrants, wide Exp) ----
            ot_sb = ot_pool.tile([DL, S], BF16, name="ot_sb")
            for cq in range(NQ):
                etp = et_pool.tile([128, NK, HL, SQ], BF16, name="etp", tag="et")
                for ck in range(NK):
                    psw = pp_mm.tile([128, HL, SQ], FP32, name="psw", tag="mm")
                    for h in range(HL):
                        hsl = slice(h * D, (h + 1) * D)
                        nc.tensor.matmul(
                            psw[:, h, :],
                            kt_sb[hsl, ts(ck, 128)],
                            qt_sb[hsl, ts(cq, SQ)],
                            start=True,
                            stop=True,
                        )
                    nc.scalar.activation(
                        etp[:, ck, :, :],
                        psw[:, :, :],
                        mybir.ActivationFunctionType.Exp,
                        scale=0.125,
                    )
                pos = [
                    pp_av.tile([128, SQ], FP32, name=f"po{h}", tag="av")
                    for h in range(HL)
                ]
                for ck in range(NK):
                    for h in range(HL):
                        nc.tensor.matmul(
                            pos[h][: D + 1, :],
                            v1_sbs[h][:, ck, :],
                            etp[:, ck, h, :],
                            start=(ck == 0),
                            stop=(ck == NK - 1),
                        )
                for h in range(HL):
                    hsl = slice(h * D, (h + 1) * D)
                    po = pos[h]
                    rin = nrm_pool.tile([1, SQ], FP32, name="rin")
                    nc.vector.reciprocal(rin[:, :], po[D : D + 1, :])
                    rb = nrm_pool.tile([D, SQ], FP32, name="rb")
                    nc.gpsimd.partition_broadcast(rb[:, :], rin[:, :])
                    nc.vector.tensor_mul(ot_sb[hsl, ts(cq, SQ)], po[0:D, :], rb[:, :])

            # ---- fc_out partial: YT = Wo_c^T.T @ O^T (wide stores, evict
            # copies alternating DVE/ACT) ----
            for co in range(NO):
                stw = stage_pool.tile([128, NQ, SQ], BF16, name="st_fcw", tag="st", bufs=2)
                for cs in range(NQ):
                    pf = pp_mm.tile([128, SQ], FP32, name="pf_fc", tag="mm")
                    nc.tensor.matmul(
                        pf[:, :],
                        wot_sb[:, co, :],
                        ot_sb[:, ts(cs, SQ)],
                        start=True,
                        stop=True,
                    )
                    if cs % 2 == 1:
                        nc.scalar.copy(stw[:, cs, :], pf[:, :])
                    else:
                        nc.vector.tensor_copy(stw[:, cs, :], pf[:, :])
                nc.sync.dma_start(
                    yt[co * 128 : (co + 1) * 128, n * S : (n + 1) * S],
                    stw[:, :, :],
                )

        for n in range(NB):
            batch_body(n)

    nc.compile()
    nc.m = get_hw_module(nc.m)
    return nc


_NC_CACHE = None


def _get_nc():
    global _NC_CACHE
    if _NC_CACHE is None:
        _NC_CACHE = _build()
    return _NC_CACHE


def _bf(a):
    return np.ascontiguousarray(a).astype(ml_dtypes.bfloat16)


def make_in_maps(x, Wq, bq, Wk, bk, Wv, bv, Wo):
    xt_host = _bf(np.asarray(x, dtype=np.float32).reshape(NBS, E).T)
    in_maps = []
    for c in range(N_CORES):
        sl = slice(c * DL, (c + 1) * DL)
        in_maps.append(
            {
                "xt": xt_host,
                "wq": _bf(np.asarray(Wq)[sl].T.reshape(NJ, 128, DL).transpose(1, 0, 2)),
                "wk": _bf(np.asarray(Wk)[sl].T.reshape(NJ, 128, DL).transpose(1, 0, 2)),
                "wv": _bf(np.asarray(Wv)[sl].T.reshape(NJ, 128, DL).transpose(1, 0, 2)),
                "wot": _bf(np.ascontiguousarray(np.asarray(Wo)[:, sl].T).reshape(DL, NO, 128)),
                "bq": np.asarray(bq)[sl].reshape(DL, 1).astype(np.float32),
                "bk": np.asarray(bk)[sl].reshape(DL, 1).astype(np.float32),
                "bv": np.asarray(bv)[sl].reshape(DL, 1).astype(np.float32),
            }
        )
    return in_maps


def kernel(x, Wq, bq, Wk, bk, Wv, bv, Wo, bo):
    nc = _get_nc()
    in_maps = make_in_maps(x, Wq, bq, Wk, bk, Wv, bv, Wo)
    res = run_bass_kernel_spmd(nc, in_maps, list(range(N_CORES)))
    yt_sum = res.results[0]["yt"].astype(np.float32)
    for c in range(1, N_CORES):
        yt_sum = yt_sum + res.results[c]["yt"].astype(np.float32)
    out = yt_sum.T + np.asarray(bo, dtype=np.float32)
    return np.ascontiguousarray(out.reshape(NB, S, E), dtype=np.float32)

